# revision 11
# baseline (speedup 1.0000x reference)
"""Trainium2 Bass kernel: batched attention-distribution forward.

Computes, for x:[B,S,F], Wq/Wk:[F,D], bq/bk:[D]:
    q = x@Wq + bq ; k = x@Wk + bk
    qkt = q @ k^T                    # [B,S,S]
    dist = softmax(qkt / rowmax(qkt))

Sharding: 8 NeuronCores, core c -> batch c//2, query-row half c%2.
Each core emits a [2048, 4096] slab; output is written bf16 (host
upcasts to f32) which halves the dominant HBM write vs f32.

Per-core pipeline, per 128-row tile (two-pass softmax, LOOKAHEAD=2):
  Pass A (tile u): 8 matmuls (bf16 in, f32 PSUM). Row max in two
    2048-wide DVE ops (only one non-PSUM operand is legal per op):
    tensor_scalar_max copies cols [0,2048) to SBUF, then a fused
    tensor_tensor_reduce maxes it against cols [2048,4096) and row-
    reduces in the same pass -> 1/M.
  Pass B (tile v = u-LOOKAHEAD): recompute qkt, exp immediately per
    1024-col chunk with the known 1/M on ACT (scale=1/M, bias=-1,
    accum_out=partial sums).
  POST (tile v-1, one step delayed so DVE never stalls on this step's
    exps): sum -> 1/sum, then normalize spread across the two engines
    that have slack (ACT mul for cols [0,1024), GPSIMD for the rest —
    DVE is the pacer at ~4.9us/tile and gets none), bf16 DMA per span.

Host-side prep is layout only (transpose x to [F,S], append a ones-row
so the bias rides inside the matmul contraction, pre-round to bf16);
every FLOP runs on device.
"""

from contextlib import ExitStack

import ml_dtypes
import numpy as np

import concourse.bacc as bacc
import concourse.bass as bass
import concourse.mybir as mybir
import concourse.tile as tile
from concourse.bass_utils import run_bass_kernel_spmd

B, S, F, D = 4, 4096, 33, 64
NCORES = 8
HALF = S // 2        # query rows per core
PT = 128             # rows per tile
NT = HALF // PT      # 16 tiles
FA = F + 1           # features + ones-row (bias folded into matmul)

F32 = mybir.dt.float32
BF16 = mybir.dt.bfloat16
NEG_BIG = -3.0e38
# normalize-engine split: cols [0,ACT_SPAN) on ACT, [ACT_SPAN,POOL_SPAN) on
# GPSIMD, [POOL_SPAN,S) on DVE. (ACT_SPAN=0, POOL_SPAN=0 -> all on DVE.)
ACT_SPAN = 1024
POOL_SPAN = 4096


def build_bass(repeat: int = 1) -> bass.Bass:
    nc = bacc.Bacc(trn_type="TRN2")
    # Packed inputs: one DMA per tensor.
    # xaw = [x[b]^T aug | Wk aug] ; xqw = [x[b]^T aug (this half) | Wq aug]
    xaw = nc.declare_dram_parameter("xaw", [FA, S + D], BF16, isOutput=False)
    xqw = nc.declare_dram_parameter("xqw", [FA, HALF + D], BF16, isOutput=False)
    out = nc.declare_dram_parameter("out", [HALF, S], BF16, isOutput=True)

    Exp = mybir.ActivationFunctionType.Exp
    Max = mybir.AluOpType.max

    with tile.TileContext(nc) as tc, ExitStack() as ctx:
        singles = ctx.enter_context(tc.tile_pool(name="singles", bufs=1))
        psum = ctx.enter_context(tc.tile_pool(name="psum", bufs=1, space="PSUM"))
        e_pool = ctx.enter_context(tc.tile_pool(name="e", bufs=3))
        o_pool = ctx.enter_context(tc.tile_pool(name="o", bufs=3))
        scr_pool = ctx.enter_context(tc.tile_pool(name="scr", bufs=2))
        stats = ctx.enter_context(tc.tile_pool(name="stats", bufs=8))

        # ---- load inputs ----
        xaw_sb = singles.tile([FA, S + D], BF16)
        nc.sync.dma_start(out=xaw_sb[:, :], in_=xaw[:, :])
        xqw_sb = singles.tile([FA, HALF + D], BF16)
        nc.sync.dma_start(out=xqw_sb[:, :], in_=xqw[:, :])
        neg1 = singles.tile([PT, 1], F32)
        nc.vector.memset(neg1[:, :], -1.0)

        # one tensor spanning all of PSUM; sliced at bank granularity
        big = psum.tile([PT, S], F32)

        # ---- projections: qT = (xq^T @ Wq)^T, kT likewise (bf16) ----
        qT = singles.tile([D, HALF], BF16)
        kT = singles.tile([D, S], BF16)

        # qT first half first (tiles 0-7 need it), then kT (tile 0 needs all
        # of it), then qT second half. PSUM ranges rotate; copies alternate
        # DVE/ACT so the prologue isn't serialized on one engine.
        def proj(psum_c0, lhsT, rhs_sb, rhs_c0, dst, dst_c0, eng):
            for j in range(2):
                nc.tensor.matmul(
                    big[0:D, psum_c0 + j * 512:psum_c0 + (j + 1) * 512],
                    lhsT=lhsT,
                    rhs=rhs_sb[:, rhs_c0 + j * 512:rhs_c0 + (j + 1) * 512],
                    start=True, stop=True,
                )
            src = big[0:D, psum_c0:psum_c0 + 1024]
            if eng == "v":
                nc.vector.tensor_copy(dst[:, dst_c0:dst_c0 + 1024], src)
            else:
                nc.scalar.copy(dst[:, dst_c0:dst_c0 + 1024], src)

        wq_l = xqw_sb[:, HALF:HALF + D]
        wk_l = xaw_sb[:, S:S + D]
        # Only what pass-A(tile 0, chunk 0) needs runs up front; the other
        # projections interleave into step 0 so the pipeline starts ~5us
        # earlier. Timing builds (repeat > 1) keep the full up-front
        # prologue: re-projecting inside the For_i would overwrite kT while
        # the previous repetition's pass-B still reads it.
        proj(3072, wq_l, xqw_sb, 0, qT, 0, "v")       # qT half 0
        proj(2048, wk_l, xaw_sb, 0, kT, 0, "s")       # kT chunk 0
        if repeat > 1:
            proj(1024, wk_l, xaw_sb, 1024, kT, 1024, "v")
            proj(0, wk_l, xaw_sb, 2048, kT, 2048, "s")
            proj(1024, wk_l, xaw_sb, 3072, kT, 3072, "v")
            proj(0, wq_l, xqw_sb, 1024, qT, 1024, "s")

        # ---- main loop: software-pipelined two-pass softmax ----
        # Steady-state per PSUM bank range: exp(v-1) -> A-mm(u) -> TTR(u)
        # -> B-mm(v) -> exp(v), staggered across the four 1024-col ranges.
        # The POST chain of tile v-1 (sum, normalize, DMA) is emitted on
        # DVE after this step's TTRs; its deps completed last step, so DVE
        # streams without waiting on this step's ACT.
        LOOKAHEAD = 2
        rep_ctx = tc.For_i(0, repeat, 1) if repeat > 1 else None
        if rep_ctx is not None:
            ctx.enter_context(rep_ctx)
        rM_of = {}
        pend = {}
        for step in range(NT + LOOKAHEAD + 1):
            u = step
            v = step - LOOKAHEAD
            w = v - 1
            if u < NT:
                lhsT = qT[:, u * PT:(u + 1) * PT]
                for c in range(4):
                    if step == 0 and repeat == 1 and c >= 1:
                        # stream the remaining kT projections in just before
                        # the first tile's chunk that needs them, using PSUM
                        # ranges this step has already drained
                        pr = {1: 3072, 2: 2048, 3: 0}[c]
                        eng = {1: "v", 2: "s", 3: "v"}[c]
                        proj(pr, wk_l, xaw_sb, c * 1024, kT, c * 1024, eng)
                    for j in range(2):
                        c0 = c * 1024 + j * 512
                        nc.tensor.matmul(
                            big[:, c0:c0 + 512],
                            lhsT=lhsT,
                            rhs=kT[:, c0:c0 + 512],
                            start=True, stop=True,
                        )
                    # row max in two 2048-wide reduce_max ops (fewer PSUM
                    # init penalties than 4x1024 + combine). tensor_tensor_
                    # reduce with op=max wedges TRN2 silicon - don't.
                    if c == 1:
                        mvec = stats.tile([PT, 2], F32, tag="mvec")
                        nc.vector.reduce_max(
                            mvec[:, 0:1], big[:, 0:2048],
                            axis=mybir.AxisListType.X,
                        )
                    if c == 3:
                        nc.vector.reduce_max(
                            mvec[:, 1:2], big[:, 2048:4096],
                            axis=mybir.AxisListType.X,
                        )
                        m1 = stats.tile([PT, 1], F32, tag="m1")
                        nc.vector.reduce_max(
                            m1[:, 0:1], mvec[:, :], axis=mybir.AxisListType.X
                        )
                if step == 0 and repeat == 1:
                    proj(1024, wq_l, xqw_sb, 1024, qT, 1024, "s")
                with tc.high_priority(offset=24):
                    rM = stats.tile([PT, 1], F32, tag="rM")
                    nc.vector.reciprocal(rM[:, 0:1], m1[:, 0:1])
                rM_of[u] = rM

            if v >= 0 and v < NT:
                lhsT = qT[:, v * PT:(v + 1) * PT]
                rM = rM_of.pop(v)
                e = e_pool.tile([PT, S], BF16)
                svec = stats.tile([PT, 4], F32, tag="svec")
                for c in range(4):
                    for j in range(2):
                        c0 = c * 1024 + j * 512
                        nc.tensor.matmul(
                            big[:, c0:c0 + 512],
                            lhsT=lhsT,
                            rhs=kT[:, c0:c0 + 512],
                            start=True, stop=True,
                        )
                    nc.scalar.activation(
                        out=e[:, c * 1024:(c + 1) * 1024],
                        in_=big[:, c * 1024:(c + 1) * 1024],
                        func=Exp,
                        bias=neg1[:, 0:1],
                        scale=rM[:, 0:1],
                        accum_out=svec[:, c:c + 1],
                    )
                pend[v] = (e, svec)

            if w >= 0:
                # POST chain for tile w: everything it reads finished last
                # step, so this never stalls the DVE stream. DVE only does
                # the tiny sum/reciprocal; the normalize runs on ACT and
                # GPSIMD, which have slack under the DVE max-scan pace.
                e, svec = pend.pop(w)
                ssum = stats.tile([PT, 1], F32, tag="ssum")
                nc.vector.reduce_sum(
                    ssum[:, 0:1], svec[:, :], axis=mybir.AxisListType.X
                )
                rs = stats.tile([PT, 1], F32, tag="rs")
                nc.vector.reciprocal(rs[:, 0:1], ssum[:, 0:1])
                o = o_pool.tile([PT, S], BF16)
                r0 = w * PT
                if ACT_SPAN > 0:
                    nc.scalar.mul(o[:, 0:ACT_SPAN], e[:, 0:ACT_SPAN], rs[:, 0:1])
                    nc.sync.dma_start(
                        out=out[r0:r0 + PT, 0:ACT_SPAN], in_=o[:, 0:ACT_SPAN]
                    )
                if POOL_SPAN > ACT_SPAN:
                    nc.gpsimd.tensor_scalar_mul(
                        o[:, ACT_SPAN:POOL_SPAN],
                        e[:, ACT_SPAN:POOL_SPAN],
                        rs[:, 0:1],
                    )
                    nc.sync.dma_start(
                        out=out[r0:r0 + PT, ACT_SPAN:POOL_SPAN],
                        in_=o[:, ACT_SPAN:POOL_SPAN],
                    )
                if S > POOL_SPAN:
                    nc.vector.tensor_scalar_mul(
                        o[:, POOL_SPAN:S], e[:, POOL_SPAN:S], rs[:, 0:1]
                    )
                    nc.sync.dma_start(
                        out=out[r0:r0 + PT, POOL_SPAN:S], in_=o[:, POOL_SPAN:S]
                    )

    nc.compile()
    return nc


_NC = None


def _get_nc() -> bass.Bass:
    global _NC
    if _NC is None:
        _NC = build_bass()
    return _NC


_NC_TIMED = {}


def _get_nc_timed(repeat: int) -> bass.Bass:
    if repeat not in _NC_TIMED:
        _NC_TIMED[repeat] = build_bass(repeat)
    return _NC_TIMED[repeat]


def prepare_in_maps(inputs: dict) -> list[dict]:
    x = np.ascontiguousarray(np.asarray(inputs["x"], dtype=np.float32))
    Wq = np.asarray(inputs["Wq"], dtype=np.float32)
    bq = np.asarray(inputs["bq"], dtype=np.float32)
    Wk = np.asarray(inputs["Wk"], dtype=np.float32)
    bk = np.asarray(inputs["bk"], dtype=np.float32)

    wq_aug = np.concatenate([Wq, bq[None, :]], axis=0)
    wk_aug = np.concatenate([Wk, bk[None, :]], axis=0)

    in_maps = []
    xaw_cache = {}
    for c in range(NCORES):
        b, h = c // 2, c % 2
        if b not in xaw_cache:
            xaw = np.empty((FA, S + D), ml_dtypes.bfloat16)
            xaw[:F, :S] = x[b].T
            xaw[F, :S] = 1.0
            xaw[:, S:] = wk_aug
            xaw_cache[b] = xaw
        xaw = xaw_cache[b]
        xqw = np.empty((FA, HALF + D), ml_dtypes.bfloat16)
        xqw[:, :HALF] = xaw[:, h * HALF:(h + 1) * HALF]
        xqw[:, HALF:] = wq_aug
        in_maps.append({"xaw": xaw, "xqw": xqw})
    return in_maps


def run(in_maps: list[dict], **kwargs):
    return run_bass_kernel_spmd(
        _get_nc(), in_maps, core_ids=list(range(NCORES)), **kwargs
    )


def assemble(results: list[dict]) -> np.ndarray:
    out = np.empty((B, S, S), np.float32)
    for c in range(NCORES):
        b, h = c // 2, c % 2
        out[b, h * HALF:(h + 1) * HALF, :] = results[c]["out"]
    return out


def kernel(**inputs) -> np.ndarray:
    res = run(prepare_in_maps(inputs))
    return assemble(res.results)


# revision 12
# speedup vs baseline: 4.6444x; 4.6444x over previous
"""Trainium2 Bass kernel: batched attention-distribution forward.

Computes, for x:[B,S,F], Wq/Wk:[F,D], bq/bk:[D]:
    q = x@Wq + bq ; k = x@Wk + bk
    qkt = q @ k^T                    # [B,S,S]
    dist = softmax(qkt / rowmax(qkt))

Sharding: 8 NeuronCores, core c -> batch c//2, query-row half c%2.
Each core emits a [2048, 4096] slab; output is written bf16 (host
upcasts to f32) which halves the dominant HBM write vs f32.

Per-core pipeline, per 128-row tile (two-pass softmax, LOOKAHEAD=2):
  Pass A (tile u): 8 matmuls (bf16 in, f32 PSUM). Row max in two
    2048-wide DVE ops (only one non-PSUM operand is legal per op):
    tensor_scalar_max copies cols [0,2048) to SBUF, then a fused
    tensor_tensor_reduce maxes it against cols [2048,4096) and row-
    reduces in the same pass -> 1/M.
  Pass B (tile v = u-LOOKAHEAD): recompute qkt, exp immediately per
    1024-col chunk with the known 1/M on ACT (scale=1/M, bias=-1,
    accum_out=partial sums).
  POST (tile v-1, one step delayed so DVE never stalls on this step's
    exps): sum -> 1/sum, then normalize spread across the two engines
    that have slack (ACT mul for cols [0,1024), GPSIMD for the rest —
    DVE is the pacer at ~4.9us/tile and gets none), bf16 DMA per span.

Host-side prep is layout only (transpose x to [F,S], append a ones-row
so the bias rides inside the matmul contraction, pre-round to bf16);
every FLOP runs on device.
"""

from contextlib import ExitStack

import ml_dtypes
import numpy as np

import concourse.bacc as bacc
import concourse.bass as bass
import concourse.mybir as mybir
import concourse.tile as tile
from concourse.bass_utils import run_bass_kernel_spmd

B, S, F, D = 4, 4096, 33, 64
NCORES = 8
HALF = S // 2        # query rows per core
PT = 128             # rows per tile
NT = HALF // PT      # 16 tiles
FA = F + 1           # features + ones-row (bias folded into matmul)

F32 = mybir.dt.float32
BF16 = mybir.dt.bfloat16
NEG_BIG = -3.0e38
# normalize-engine split: cols [0,ACT_SPAN) on ACT, [ACT_SPAN,POOL_SPAN) on
# GPSIMD, [POOL_SPAN,S) on DVE. GPSIMD measured ~20 ns/elem (30x below its
# cost model) - keep it out (POOL_SPAN == ACT_SPAN).
ACT_SPAN = 2048
POOL_SPAN = 2048


def build_bass(repeat: int = 1) -> bass.Bass:
    nc = bacc.Bacc(trn_type="TRN2")
    # Packed inputs: one DMA per tensor.
    # xaw = [x[b]^T aug | Wk aug] ; xqw = [x[b]^T aug (this half) | Wq aug]
    xaw = nc.declare_dram_parameter("xaw", [FA, S + D], BF16, isOutput=False)
    xqw = nc.declare_dram_parameter("xqw", [FA, HALF + D], BF16, isOutput=False)
    out = nc.declare_dram_parameter("out", [HALF, S], BF16, isOutput=True)

    Exp = mybir.ActivationFunctionType.Exp
    Max = mybir.AluOpType.max

    with tile.TileContext(nc) as tc, ExitStack() as ctx:
        singles = ctx.enter_context(tc.tile_pool(name="singles", bufs=1))
        psum = ctx.enter_context(tc.tile_pool(name="psum", bufs=1, space="PSUM"))
        e_pool = ctx.enter_context(tc.tile_pool(name="e", bufs=3))
        o_pool = ctx.enter_context(tc.tile_pool(name="o", bufs=3))
        scr_pool = ctx.enter_context(tc.tile_pool(name="scr", bufs=2))
        stats = ctx.enter_context(tc.tile_pool(name="stats", bufs=8))

        # ---- load inputs ----
        xaw_sb = singles.tile([FA, S + D], BF16)
        nc.sync.dma_start(out=xaw_sb[:, :], in_=xaw[:, :])
        xqw_sb = singles.tile([FA, HALF + D], BF16)
        nc.sync.dma_start(out=xqw_sb[:, :], in_=xqw[:, :])
        neg1 = singles.tile([PT, 1], F32)
        nc.vector.memset(neg1[:, :], -1.0)

        # one tensor spanning all of PSUM; sliced at bank granularity
        big = psum.tile([PT, S], F32)

        # ---- projections: qT = (xq^T @ Wq)^T, kT likewise (bf16) ----
        qT = singles.tile([D, HALF], BF16)
        kT = singles.tile([D, S], BF16)

        # qT first half first (tiles 0-7 need it), then kT (tile 0 needs all
        # of it), then qT second half. PSUM ranges rotate; copies alternate
        # DVE/ACT so the prologue isn't serialized on one engine.
        def proj(psum_c0, lhsT, rhs_sb, rhs_c0, dst, dst_c0, eng):
            for j in range(2):
                nc.tensor.matmul(
                    big[0:D, psum_c0 + j * 512:psum_c0 + (j + 1) * 512],
                    lhsT=lhsT,
                    rhs=rhs_sb[:, rhs_c0 + j * 512:rhs_c0 + (j + 1) * 512],
                    start=True, stop=True,
                )
            src = big[0:D, psum_c0:psum_c0 + 1024]
            if eng == "v":
                nc.vector.tensor_copy(dst[:, dst_c0:dst_c0 + 1024], src)
            else:
                nc.scalar.copy(dst[:, dst_c0:dst_c0 + 1024], src)

        wq_l = xqw_sb[:, HALF:HALF + D]
        wk_l = xaw_sb[:, S:S + D]
        # Only what pass-A(tile 0, chunk 0) needs runs up front; the other
        # projections interleave into step 0 so the pipeline starts ~5us
        # earlier. Timing builds (repeat > 1) keep the full up-front
        # prologue: re-projecting inside the For_i would overwrite kT while
        # the previous repetition's pass-B still reads it.
        proj(3072, wq_l, xqw_sb, 0, qT, 0, "v")       # qT half 0
        proj(2048, wk_l, xaw_sb, 0, kT, 0, "s")       # kT chunk 0
        if repeat > 1:
            proj(1024, wk_l, xaw_sb, 1024, kT, 1024, "v")
            proj(0, wk_l, xaw_sb, 2048, kT, 2048, "s")
            proj(1024, wk_l, xaw_sb, 3072, kT, 3072, "v")
            proj(0, wq_l, xqw_sb, 1024, qT, 1024, "s")

        # ---- main loop: software-pipelined two-pass softmax ----
        # Steady-state per PSUM bank range: exp(v-1) -> A-mm(u) -> TTR(u)
        # -> B-mm(v) -> exp(v), staggered across the four 1024-col ranges.
        # The POST chain of tile v-1 (sum, normalize, DMA) is emitted on
        # DVE after this step's TTRs; its deps completed last step, so DVE
        # streams without waiting on this step's ACT.
        LOOKAHEAD = 2
        rep_ctx = tc.For_i(0, repeat, 1) if repeat > 1 else None
        if rep_ctx is not None:
            ctx.enter_context(rep_ctx)
        rM_of = {}
        pend = {}
        for step in range(NT + LOOKAHEAD + 1):
            u = step
            v = step - LOOKAHEAD
            w = v - 1
            if u < NT:
                lhsT = qT[:, u * PT:(u + 1) * PT]
                for c in range(4):
                    if step == 0 and repeat == 1 and c >= 1:
                        # stream the remaining kT projections in just before
                        # the first tile's chunk that needs them, using PSUM
                        # ranges this step has already drained
                        pr = {1: 3072, 2: 2048, 3: 0}[c]
                        eng = {1: "v", 2: "s", 3: "v"}[c]
                        proj(pr, wk_l, xaw_sb, c * 1024, kT, c * 1024, eng)
                    for j in range(2):
                        c0 = c * 1024 + j * 512
                        nc.tensor.matmul(
                            big[:, c0:c0 + 512],
                            lhsT=lhsT,
                            rhs=kT[:, c0:c0 + 512],
                            start=True, stop=True,
                        )
                    # row max in two 2048-wide reduce_max ops (fewer PSUM
                    # init penalties than 4x1024 + combine). tensor_tensor_
                    # reduce with op=max wedges TRN2 silicon - don't.
                    if c == 1:
                        mvec = stats.tile([PT, 2], F32, tag="mvec")
                        nc.vector.reduce_max(
                            mvec[:, 0:1], big[:, 0:2048],
                            axis=mybir.AxisListType.X,
                        )
                    if c == 3:
                        nc.vector.reduce_max(
                            mvec[:, 1:2], big[:, 2048:4096],
                            axis=mybir.AxisListType.X,
                        )
                        m1 = stats.tile([PT, 1], F32, tag="m1")
                        nc.vector.reduce_max(
                            m1[:, 0:1], mvec[:, :], axis=mybir.AxisListType.X
                        )
                if step == 0 and repeat == 1:
                    proj(1024, wq_l, xqw_sb, 1024, qT, 1024, "s")
                with tc.high_priority(offset=24):
                    rM = stats.tile([PT, 1], F32, tag="rM")
                    nc.vector.reciprocal(rM[:, 0:1], m1[:, 0:1])
                rM_of[u] = rM

            if v >= 0 and v < NT:
                lhsT = qT[:, v * PT:(v + 1) * PT]
                rM = rM_of.pop(v)
                e = e_pool.tile([PT, S], BF16)
                svec = stats.tile([PT, 4], F32, tag="svec")
                for c in range(4):
                    for j in range(2):
                        c0 = c * 1024 + j * 512
                        nc.tensor.matmul(
                            big[:, c0:c0 + 512],
                            lhsT=lhsT,
                            rhs=kT[:, c0:c0 + 512],
                            start=True, stop=True,
                        )
                    nc.scalar.activation(
                        out=e[:, c * 1024:(c + 1) * 1024],
                        in_=big[:, c * 1024:(c + 1) * 1024],
                        func=Exp,
                        bias=neg1[:, 0:1],
                        scale=rM[:, 0:1],
                        accum_out=svec[:, c:c + 1],
                    )
                pend[v] = (e, svec)

            if w >= 0:
                # POST chain for tile w: everything it reads finished last
                # step, so this never stalls the DVE stream. DVE only does
                # the tiny sum/reciprocal; the normalize runs on ACT and
                # GPSIMD, which have slack under the DVE max-scan pace.
                e, svec = pend.pop(w)
                ssum = stats.tile([PT, 1], F32, tag="ssum")
                nc.vector.reduce_sum(
                    ssum[:, 0:1], svec[:, :], axis=mybir.AxisListType.X
                )
                rs = stats.tile([PT, 1], F32, tag="rs")
                nc.vector.reciprocal(rs[:, 0:1], ssum[:, 0:1])
                o = o_pool.tile([PT, S], BF16)
                r0 = w * PT
                if ACT_SPAN > 0:
                    nc.scalar.mul(o[:, 0:ACT_SPAN], e[:, 0:ACT_SPAN], rs[:, 0:1])
                    nc.sync.dma_start(
                        out=out[r0:r0 + PT, 0:ACT_SPAN], in_=o[:, 0:ACT_SPAN]
                    )
                if POOL_SPAN > ACT_SPAN:
                    nc.gpsimd.tensor_scalar_mul(
                        o[:, ACT_SPAN:POOL_SPAN],
                        e[:, ACT_SPAN:POOL_SPAN],
                        rs[:, 0:1],
                    )
                    nc.sync.dma_start(
                        out=out[r0:r0 + PT, ACT_SPAN:POOL_SPAN],
                        in_=o[:, ACT_SPAN:POOL_SPAN],
                    )
                if S > POOL_SPAN:
                    nc.vector.tensor_scalar_mul(
                        o[:, POOL_SPAN:S], e[:, POOL_SPAN:S], rs[:, 0:1]
                    )
                    nc.sync.dma_start(
                        out=out[r0:r0 + PT, POOL_SPAN:S], in_=o[:, POOL_SPAN:S]
                    )

    nc.compile()
    return nc


_NC = None


def _get_nc() -> bass.Bass:
    global _NC
    if _NC is None:
        _NC = build_bass()
    return _NC


_NC_TIMED = {}


def _get_nc_timed(repeat: int) -> bass.Bass:
    if repeat not in _NC_TIMED:
        _NC_TIMED[repeat] = build_bass(repeat)
    return _NC_TIMED[repeat]


def prepare_in_maps(inputs: dict) -> list[dict]:
    x = np.ascontiguousarray(np.asarray(inputs["x"], dtype=np.float32))
    Wq = np.asarray(inputs["Wq"], dtype=np.float32)
    bq = np.asarray(inputs["bq"], dtype=np.float32)
    Wk = np.asarray(inputs["Wk"], dtype=np.float32)
    bk = np.asarray(inputs["bk"], dtype=np.float32)

    wq_aug = np.concatenate([Wq, bq[None, :]], axis=0)
    wk_aug = np.concatenate([Wk, bk[None, :]], axis=0)

    in_maps = []
    xaw_cache = {}
    for c in range(NCORES):
        b, h = c // 2, c % 2
        if b not in xaw_cache:
            xaw = np.empty((FA, S + D), ml_dtypes.bfloat16)
            xaw[:F, :S] = x[b].T
            xaw[F, :S] = 1.0
            xaw[:, S:] = wk_aug
            xaw_cache[b] = xaw
        xaw = xaw_cache[b]
        xqw = np.empty((FA, HALF + D), ml_dtypes.bfloat16)
        xqw[:, :HALF] = xaw[:, h * HALF:(h + 1) * HALF]
        xqw[:, HALF:] = wq_aug
        in_maps.append({"xaw": xaw, "xqw": xqw})
    return in_maps


def run(in_maps: list[dict], **kwargs):
    return run_bass_kernel_spmd(
        _get_nc(), in_maps, core_ids=list(range(NCORES)), **kwargs
    )


def assemble(results: list[dict]) -> np.ndarray:
    out = np.empty((B, S, S), np.float32)
    for c in range(NCORES):
        b, h = c // 2, c % 2
        out[b, h * HALF:(h + 1) * HALF, :] = results[c]["out"]
    return out


def kernel(**inputs) -> np.ndarray:
    res = run(prepare_in_maps(inputs))
    return assemble(res.results)


# revision 22
# speedup vs baseline: 5.1739x; 1.1140x over previous
"""Trainium2 Bass kernel: batched attention-distribution forward.

Computes, for x:[B,S,F], Wq/Wk:[F,D], bq/bk:[D]:
    q = x@Wq + bq ; k = x@Wk + bk
    qkt = q @ k^T                    # [B,S,S]
    dist = softmax(qkt / rowmax(qkt))

Sharding: 8 NeuronCores, core c -> batch c//2, query-row half c%2.
Each core emits a [2048, 4096] slab; output is written bf16 (host
upcasts to f32) which halves the dominant HBM write vs f32.

Two-pass softmax over one shared [128,4096] PSUM tensor, bank-granular
pipelining (steady-state per 1024-col range: exp(v-1) -> A-mm(u) ->
reduce_max(u) -> B-mm(v) -> exp(v)):
  Pass A (tile u): 8 matmuls; DVE reduce_max per 2048-col half ->
    combine -> 1/M. (tensor_tensor_reduce with op=max wedges TRN2
    silicon; GPSIMD streams at ~20 ns/elem; both measured on HW - so
    the scan lives on DVE at its port limit of 1 f32/cycle.)
  Pass B (tile v = u-LOOKAHEAD): recompute qkt, exp per 1024-col chunk
    with the known 1/M on ACT (scale=1/M, bias=-1, accum_out=sums).
  POST (tile v-1, one step delayed so DVE never stalls on this step's
    exps): sum -> 1/sum -> normalize on DVE in 4x bf16 mode -> one
    1 MiB bf16 DMA.

Engine budget per 128-row tile: DVE ~5.4us (the pacer), ACT ~4.2us,
PE ~3.5us, DMA ~2.9us.

Host-side prep is layout only (transpose x to [F,S], append a ones-row
so the bias rides inside the matmul contraction, pre-round to bf16);
every FLOP runs on device.
"""

from contextlib import ExitStack

import ml_dtypes
import numpy as np

import concourse.bacc as bacc
import concourse.bass as bass
import concourse.mybir as mybir
import concourse.tile as tile
from concourse.bass_utils import run_bass_kernel_spmd

B, S, F, D = 4, 4096, 33, 64
NCORES = 8
HALF = S // 2        # query rows per core
PT = 128             # rows per tile
NT = HALF // PT      # 16 tiles
FA = F + 1           # features + ones-row (bias folded into matmul)

F32 = mybir.dt.float32
BF16 = mybir.dt.bfloat16

LOOKAHEAD = 1      # steps between pass A (max) and pass B (exp)
POST_DELAY = 1     # steps between exp and its normalize/DMA chain
ACT_SPAN = 0       # leading cols normalized on ACT (rest on DVE)
POOL_MICROS = False  # run mvec/svec combines on GPSIMD


def build_bass(repeat: int = 1) -> bass.Bass:
    nc = bacc.Bacc(trn_type="TRN2")
    # Packed inputs: one DMA per tensor.
    # xaw = [x[b]^T aug | Wk aug] ; xqw = [x[b]^T aug (this half) | Wq aug]
    xaw = nc.declare_dram_parameter("xaw", [FA, S + D], BF16, isOutput=False)
    xqw = nc.declare_dram_parameter("xqw", [FA, HALF + D], BF16, isOutput=False)
    out = nc.declare_dram_parameter("out", [HALF, S], BF16, isOutput=True)

    Exp = mybir.ActivationFunctionType.Exp

    with tile.TileContext(nc) as tc, ExitStack() as ctx:
        singles = ctx.enter_context(tc.tile_pool(name="singles", bufs=1))
        psum = ctx.enter_context(tc.tile_pool(name="psum", bufs=1, space="PSUM"))
        e_pool = ctx.enter_context(tc.tile_pool(name="e", bufs=3))
        o_pool = ctx.enter_context(tc.tile_pool(name="o", bufs=3))
        stats = ctx.enter_context(tc.tile_pool(name="stats", bufs=8))

        # ---- load inputs ----
        xaw_sb = singles.tile([FA, S + D], BF16)
        nc.sync.dma_start(out=xaw_sb[:, :], in_=xaw[:, :])
        xqw_sb = singles.tile([FA, HALF + D], BF16)
        nc.sync.dma_start(out=xqw_sb[:, :], in_=xqw[:, :])
        neg1 = singles.tile([PT, 1], F32)
        nc.vector.memset(neg1[:, :], -1.0)

        # one tensor spanning all of PSUM; sliced at bank granularity
        big = psum.tile([PT, S], F32)

        # ---- projections: qT = (xq^T @ Wq)^T, kT likewise (bf16) ----
        qT = singles.tile([D, HALF], BF16)
        kT = singles.tile([D, S], BF16)

        # qT first half first (tiles 0-7 need it), then kT (tile 0 needs all
        # of it), then qT second half. PSUM ranges rotate; copies alternate
        # DVE/ACT so the prologue isn't serialized on one engine.
        def proj(psum_c0, lhsT, rhs_sb, rhs_c0, dst, dst_c0, eng):
            for j in range(2):
                nc.tensor.matmul(
                    big[0:D, psum_c0 + j * 512:psum_c0 + (j + 1) * 512],
                    lhsT=lhsT,
                    rhs=rhs_sb[:, rhs_c0 + j * 512:rhs_c0 + (j + 1) * 512],
                    start=True, stop=True,
                )
            src = big[0:D, psum_c0:psum_c0 + 1024]
            if eng == "v":
                nc.vector.tensor_copy(dst[:, dst_c0:dst_c0 + 1024], src)
            else:
                nc.scalar.copy(dst[:, dst_c0:dst_c0 + 1024], src)

        wq_l = xqw_sb[:, HALF:HALF + D]
        wk_l = xaw_sb[:, S:S + D]
        # Only what pass-A(tile 0) needs first runs up front; the rest
        # interleave into step 0 using PSUM ranges it has already drained.
        # Timing builds (repeat > 1) keep the full up-front prologue:
        # re-projecting inside the For_i would overwrite kT while the
        # previous repetition's pass-B still reads it.
        proj(3072, wq_l, xqw_sb, 0, qT, 0, "v")       # qT half 0
        proj(2048, wk_l, xaw_sb, 0, kT, 0, "s")       # kT chunk 0
        if repeat > 1:
            proj(1024, wk_l, xaw_sb, 1024, kT, 1024, "v")
            proj(0, wk_l, xaw_sb, 2048, kT, 2048, "s")
            proj(1024, wk_l, xaw_sb, 3072, kT, 3072, "v")
            proj(0, wq_l, xqw_sb, 1024, qT, 1024, "s")

        # ---- main loop: software-pipelined two-pass softmax ----
        rep_ctx = tc.For_i(0, repeat, 1) if repeat > 1 else None
        if rep_ctx is not None:
            ctx.enter_context(rep_ctx)
        rM_of = {}
        pend = {}
        for step in range(NT + LOOKAHEAD + POST_DELAY):
            u = step
            v = step - LOOKAHEAD
            w = v - POST_DELAY

            if w >= 0:
                # POST head for tile w: emitted FIRST so the sum/reciprocal
                # and the ACT-span normalize fill the engines' early-step
                # idle (everything it reads finished last step).
                e_w, svec_w = pend.pop(w)
                ssum = stats.tile([PT, 1], F32, tag="ssum")
                nc.vector.reduce_sum(
                    ssum[:, 0:1], svec_w[:, :], axis=mybir.AxisListType.X
                )
                rs = stats.tile([PT, 1], F32, tag="rs")
                nc.vector.reciprocal(rs[:, 0:1], ssum[:, 0:1])
                o = o_pool.tile([PT, S], BF16)
                r0 = w * PT
                if ACT_SPAN > 0:
                    nc.scalar.mul(
                        o[:, 0:ACT_SPAN], e_w[:, 0:ACT_SPAN], rs[:, 0:1]
                    )
                    nc.sync.dma_start(
                        out=out[r0:r0 + PT, 0:ACT_SPAN], in_=o[:, 0:ACT_SPAN]
                    )

            if u < NT:
                lhsT = qT[:, u * PT:(u + 1) * PT]
                mvec = stats.tile([PT, 2], F32, tag="mvec")
                for c in range(4):
                    if step == 0 and repeat == 1 and c >= 1:
                        # stream the remaining kT projections in just before
                        # the first tile's chunk that needs them, using PSUM
                        # ranges this step has already drained
                        pr = {1: 3072, 2: 2048, 3: 0}[c]
                        eng = {1: "v", 2: "s", 3: "v"}[c]
                        proj(pr, wk_l, xaw_sb, c * 1024, kT, c * 1024, eng)
                    for j in range(2):
                        c0 = c * 1024 + j * 512
                        nc.tensor.matmul(
                            big[:, c0:c0 + 512],
                            lhsT=lhsT,
                            rhs=kT[:, c0:c0 + 512],
                            start=True, stop=True,
                        )
                    if c == 1:
                        nc.vector.reduce_max(
                            mvec[:, 0:1], big[:, 0:2048],
                            axis=mybir.AxisListType.X,
                        )
                    if c == 3:
                        nc.vector.reduce_max(
                            mvec[:, 1:2], big[:, 2048:4096],
                            axis=mybir.AxisListType.X,
                        )
                if step == 0 and repeat == 1:
                    proj(1024, wq_l, xqw_sb, 1024, qT, 1024, "s")
                with tc.high_priority(offset=24):
                    m = stats.tile([PT, 1], F32, tag="m")
                    if POOL_MICROS:
                        nc.gpsimd.reduce_max(
                            m[:, 0:1], mvec[:, :], axis=mybir.AxisListType.X
                        )
                    else:
                        nc.vector.reduce_max(
                            m[:, 0:1], mvec[:, :], axis=mybir.AxisListType.X
                        )
                    rM = stats.tile([PT, 1], F32, tag="rM")
                    nc.vector.reciprocal(rM[:, 0:1], m[:, 0:1])
                rM_of[u] = rM

            if v >= 0 and v < NT:
                lhsT = qT[:, v * PT:(v + 1) * PT]
                rM = rM_of.pop(v)
                e = e_pool.tile([PT, S], BF16)
                svec = stats.tile([PT, 4], F32, tag="svec")
                for c in range(4):
                    for j in range(2):
                        c0 = c * 1024 + j * 512
                        nc.tensor.matmul(
                            big[:, c0:c0 + 512],
                            lhsT=lhsT,
                            rhs=kT[:, c0:c0 + 512],
                            start=True, stop=True,
                        )
                    nc.scalar.activation(
                        out=e[:, c * 1024:(c + 1) * 1024],
                        in_=big[:, c * 1024:(c + 1) * 1024],
                        func=Exp,
                        bias=neg1[:, 0:1],
                        scale=rM[:, 0:1],
                        accum_out=svec[:, c:c + 1],
                    )
                pend[v] = (e, svec)

            if w >= 0:
                # POST tail for tile w: the DVE-span normalize, emitted
                # after this step's reduces so it never delays them.
                nc.vector.tensor_scalar_mul(
                    o[:, ACT_SPAN:S], e_w[:, ACT_SPAN:S], rs[:, 0:1]
                )
                nc.sync.dma_start(
                    out=out[r0:r0 + PT, ACT_SPAN:S], in_=o[:, ACT_SPAN:S]
                )

    nc.compile()
    return nc


_NC = None


def _get_nc() -> bass.Bass:
    global _NC
    if _NC is None:
        _NC = build_bass()
    return _NC


_NC_TIMED = {}


def _get_nc_timed(repeat: int) -> bass.Bass:
    if repeat not in _NC_TIMED:
        _NC_TIMED[repeat] = build_bass(repeat)
    return _NC_TIMED[repeat]


def prepare_in_maps(inputs: dict) -> list[dict]:
    x = np.ascontiguousarray(np.asarray(inputs["x"], dtype=np.float32))
    Wq = np.asarray(inputs["Wq"], dtype=np.float32)
    bq = np.asarray(inputs["bq"], dtype=np.float32)
    Wk = np.asarray(inputs["Wk"], dtype=np.float32)
    bk = np.asarray(inputs["bk"], dtype=np.float32)

    wq_aug = np.concatenate([Wq, bq[None, :]], axis=0)
    wk_aug = np.concatenate([Wk, bk[None, :]], axis=0)

    in_maps = []
    xaw_cache = {}
    for c in range(NCORES):
        b, h = c // 2, c % 2
        if b not in xaw_cache:
            xaw = np.empty((FA, S + D), ml_dtypes.bfloat16)
            xaw[:F, :S] = x[b].T
            xaw[F, :S] = 1.0
            xaw[:, S:] = wk_aug
            xaw_cache[b] = xaw
        xaw = xaw_cache[b]
        xqw = np.empty((FA, HALF + D), ml_dtypes.bfloat16)
        xqw[:, :HALF] = xaw[:, h * HALF:(h + 1) * HALF]
        xqw[:, HALF:] = wq_aug
        in_maps.append({"xaw": xaw, "xqw": xqw})
    return in_maps


def run(in_maps: list[dict], **kwargs):
    return run_bass_kernel_spmd(
        _get_nc(), in_maps, core_ids=list(range(NCORES)), **kwargs
    )


def assemble(results: list[dict]) -> np.ndarray:
    out = np.empty((B, S, S), np.float32)
    for c in range(NCORES):
        b, h = c // 2, c % 2
        out[b, h * HALF:(h + 1) * HALF, :] = results[c]["out"]
    return out


def kernel(**inputs) -> np.ndarray:
    res = run(prepare_in_maps(inputs))
    return assemble(res.results)
